# revision 13
# baseline (speedup 1.0000x reference)
"""3-layer GraphSAGE (mean agg) on 8 trn2 NeuronCores.

Sharding: nodes data-parallel (6250/core), weights replicated. A global node
relabeling (region-preserving 2-D bin packing on per-node in-degree from each
source region) assigns nodes to (core, block) so per-(block,region) edge
counts sit just under multiples of 128, cutting gather padding to ~2%.

Per core: edges with dst in its node range, grouped (group-major) by
(dst-block-group, src-region), padded to core-invariant per-(block,region)
window counts so one SPMD program works for all cores. Neighbor features
fetched by dma_gather (bf16 256B rows) spread round-robin over 4 SWDGE
queues, aggregated per 128-edge window by one-hot matmuls accumulating in
PSUM (one-hot P built in bulk per gather segment via broadcast-AP is_equal),
then fused dense layer in bf16.

Source nodes split in two regions by local row (A: rows 0..3071 per core,
B: rest) so the inter-layer AllGather runs as two chunks; chunk A can fire
mid-layer (after block 23) and overlap the remaining compute. Gathers
address hgA [8*3072, D] / hgB [8*3178, D] with int16 offsets.
"""

import os
import sys

sys.path.insert(0, "/opt/trn_rl_repo")

import numpy as np
import ml_dtypes

N_NODES = 50000
N_EDGES = 800000
DIM = 128
N_LAYERS = 3
CORES = 8
NPC = N_NODES // CORES          # 6250 nodes per core
BLK = 128
NBLK = (NPC + BLK - 1) // BLK   # 49 blocks (last has 106 valid rows)
NPC_PAD = NBLK * BLK            # 6272
ABLK = 24                       # blocks in region A (per core)
AROWS = ABLK * BLK              # 3072
BROWS = NPC - AROWS             # 3178
NA = CORES * AROWS              # 24576 rows in hgA (int16-safe < 32768)
NB = CORES * BROWS              # 25424 rows in hgB
GB = int(os.environ.get("GNN_GB", "3"))   # blocks per gather group
NQ = 4                          # SWDGE queues (ucode max)
CH = 1024                       # idx per gather inst (= ring capacity/queue)

LAST_EXEC_NS = [None]
LAST_PROFILE = [None]


def _pack_nodes(src, dst):
    """Region-preserving node relabeling: 2-D best-fit-decreasing packing of
    nodes into (core, block) bins so per-(block, src-region) in-degree sums
    sit just under multiples of 128. Returns perm (old id -> new id)."""
    l_s = src % NPC
    isA_src = l_s < AROWS
    dA = np.bincount(dst[isA_src], minlength=N_NODES).astype(np.float64)
    dB = np.bincount(dst[~isA_src], minlength=N_NODES).astype(np.float64)
    isA_node = (np.arange(N_NODES) % NPC) < AROWS

    def mkcaps(npos, sumA, sumB):
        baseA, extraA = divmod(sumA, npos)
        capA = np.full(npos, baseA)
        capA[:extraA] += 1
        baseB, extraB = divmod(sumB, npos)
        capB = np.full(npos, baseB)
        capB[npos - extraB :] += 1
        return capA * 128.0, capB * 128.0

    def pack(nodes, npos, cards, slackA, slackB):
        sumA = int(np.ceil(dA[nodes].sum() / CORES / 128)) + slackA
        sumB = int(np.ceil(dB[nodes].sum() / CORES / 128)) + slackB
        capA, capB = mkcaps(npos, sumA, sumB)
        nbins = CORES * npos
        bcapA = np.tile(capA, CORES)
        bcapB = np.tile(capB, CORES)
        bcard = np.tile(cards, CORES)
        da, db = dA[nodes], dB[nodes]
        order = np.argsort(-(da + db))
        sA = np.zeros(nbins)
        sB = np.zeros(nbins)
        cnt = np.zeros(nbins, np.int64)
        binof = np.empty(len(nodes), np.int64)
        for i in order:
            fa, fb = sA + da[i], sB + db[i]
            feas = (fa <= bcapA) & (fb <= bcapB) & (cnt < bcard)
            if feas.any():
                score = np.where(
                    feas,
                    np.maximum(fa / bcapA, fb / bcapB) - 0.3 * cnt / bcard,
                    np.inf,
                )
            else:
                score = np.where(
                    cnt < bcard, np.maximum(fa / bcapA, fb / bcapB), np.inf
                )
            j = int(np.argmin(score))
            binof[i] = j
            sA[j] += da[i]
            sB[j] += db[i]
            cnt[j] += 1
        assert (cnt == bcard).all()
        return binof

    perm = np.empty(N_NODES, np.int64)
    nodesA = np.nonzero(isA_node)[0]
    binA = pack(nodesA, ABLK, np.full(ABLK, 128), 3, 2)
    # stable order within bin
    order = np.lexsort((nodesA, binA))
    slot = np.arange(len(nodesA)) - np.searchsorted(binA[order], binA[order])
    c, p = binA[order] // ABLK, binA[order] % ABLK
    perm[nodesA[order]] = c * NPC + p * 128 + slot

    nodesB = np.nonzero(~isA_node)[0]
    nposB = NBLK - ABLK  # 25
    cardsB = np.array([128] * (nposB - 1) + [NPC - AROWS - (nposB - 1) * 128])
    binB = pack(nodesB, nposB, cardsB, 3, 2)
    order = np.lexsort((nodesB, binB))
    slot = np.arange(len(nodesB)) - np.searchsorted(binB[order], binB[order])
    c, p = binB[order] // nposB, binB[order] % nposB
    perm[nodesB[order]] = c * NPC + AROWS + p * 128 + slot
    return perm


def _src_region(src):
    """Map (permuted) src node id -> (region, offset-in-region-tensor)."""
    c = src // NPC
    l = src % NPC
    isB = l >= AROWS
    off = np.where(isB, c * BROWS + (l - AROWS), c * AROWS + l)
    return isB.astype(np.int64), off


def _preprocess(src, dst):
    """Host-side graph preprocessing on permuted ids. Returns per-core
    index/dslot arrays plus the shared (core-invariant) window schedule.
    Window stream order is group-major: for g: for r: for b in group."""
    src = np.asarray(src).astype(np.int64)
    dst = np.asarray(dst).astype(np.int64)

    owner = dst // NPC
    run, soff = _src_region(src)
    dloc = dst - owner * NPC
    blk = dloc // BLK
    grp = blk // GB

    counts = np.zeros((CORES, NBLK, 2), np.int64)
    np.add.at(counts, (owner, blk, run), 1)
    W = np.maximum(1, -(-counts.max(axis=0) // BLK))  # [NBLK, 2]

    groups = [list(range(s, min(s + GB, NBLK))) for s in range(0, NBLK, GB)]
    # group-major window offsets
    woff = np.zeros((NBLK, 2), np.int64)
    w = 0
    for g, blocks in enumerate(groups):
        for r in range(2):
            for b in blocks:
                woff[b, r] = w
                w += int(W[b, r])
    nw_total = w

    per_core = []
    for c in range(CORES):
        sel = owner == c
        es, eb, er, ed, eg = soff[sel], blk[sel], run[sel], dloc[sel], grp[sel]
        order = np.lexsort((eb, er, eg))
        es, eb, er, ed = es[order], eb[order], er[order], ed[order]

        idx_out = np.zeros(nw_total * BLK, np.int16)
        dslot_out = np.full(nw_total * BLK, 255.0, np.float32)
        epos = 0
        for g, blocks in enumerate(groups):
            for r in range(2):
                for b in blocks:
                    cnt = int(counts[c, b, r])
                    e0, e1 = epos, epos + cnt
                    o0 = int(woff[b, r]) * BLK
                    idx_out[o0 : o0 + cnt] = es[e0:e1].astype(np.int16)
                    dslot_out[o0 : o0 + cnt] = (ed[e0:e1] - b * BLK).astype(
                        np.float32
                    )
                    # pads: idx 0 (valid row, gathered but zeroed by P)
                    epos = e1
        assert epos == len(es)

        # wrap indices in 16 partitions, replicate to 128 (one copy / Q7 core)
        wrapped = idx_out.reshape(-1, 16).T.copy()        # [16, nw_total*8]
        idx128 = np.tile(wrapped, (8, 1))                 # [128, nw_total*8]
        # dslotT: [128, nw_total]; column w = dslots of window w's 128 edges
        dslotT = dslot_out.reshape(nw_total, BLK).T.copy()
        per_core.append((idx128, dslotT))

    return W, woff, nw_total, per_core


def _build_program(W, woff, nw_total):
    import concourse.bass as bass
    import concourse.mybir as mybir
    import concourse.tile as tile
    from concourse import bacc

    f32 = mybir.dt.float32
    bf16 = mybir.dt.bfloat16
    i16 = mybir.dt.int16

    nc = bacc.Bacc(
        "TRN2",
        target_bir_lowering=False,
        num_devices=CORES,
        num_swdge_queues=NQ,
        dynamic_dma_scratch_size=int(os.environ.get("GNN_SCRATCH", "32768")),
    )

    # I/O (xA/xB: layer-0 gather sources in region layout)
    xA_in = nc.declare_dram_parameter("xA", [NA, DIM], bf16, isOutput=False)
    xB_in = nc.declare_dram_parameter("xB", [NB, DIM], bf16, isOutput=False)
    xT_in = nc.declare_dram_parameter("xT", [DIM, NPC_PAD], bf16, isOutput=False)
    idx_in = nc.declare_dram_parameter("idx", [128, nw_total * 8], i16, isOutput=False)
    dslot_in = nc.declare_dram_parameter("dslot", [128, nw_total], f32, isOutput=False)
    invdeg_in = nc.declare_dram_parameter("invdeg", [1, NPC_PAD], bf16, isOutput=False)
    ws_in = nc.declare_dram_parameter("Wself", [N_LAYERS * DIM, DIM], bf16, isOutput=False)
    wn_in = nc.declare_dram_parameter("Wneigh", [N_LAYERS * DIM, DIM], bf16, isOutput=False)
    b_in = nc.declare_dram_parameter("bias", [N_LAYERS, DIM], bf16, isOutput=False)
    iota_in = nc.declare_dram_parameter("iota", [128, 128], f32, isOutput=False)
    ident_in = nc.declare_dram_parameter("ident", [128, 128], bf16, isOutput=False)
    out_ext = nc.declare_dram_parameter("out", [NPC, DIM], f32, isOutput=True)

    # internal DRAM for collectives (two chunks per boundary)
    hown = [nc.dram_tensor(f"hown{l}", [NPC, DIM], bf16) for l in range(2)]
    hgA = [
        nc.dram_tensor(f"hgA{l}", [NA, DIM], bf16, addr_space="Shared")
        for l in range(2)
    ]
    hgB = [
        nc.dram_tensor(f"hgB{l}", [NB, DIM], bf16, addr_space="Shared")
        for l in range(2)
    ]
    rg = [list(range(CORES))]
    chunk_ag = os.environ.get("GNN_CHUNK_AG", "0") == "1"

    groups = [list(range(s, min(s + GB, NBLK))) for s in range(0, NBLK, GB)]
    # idx prefix: windows of group 0 (both regions) for the early small load
    nw_g0 = int(sum(int(W[b, r]) for r in range(2) for b in groups[0]))

    qctr = [0]  # SWDGE queue round-robin across all gathers

    with tile.TileContext(nc) as tc:
        with (
            tc.tile_pool(name="persist", bufs=1) as pp,
            tc.tile_pool(name="msg", bufs=int(os.environ.get("GNN_MSGBUFS", "4"))) as msgp,
            tc.tile_pool(name="pwin", bufs=int(os.environ.get("GNN_PBUFS", "3"))) as pwp,
            tc.tile_pool(name="work", bufs=4) as wkp,
            tc.tile_pool(name="psA", bufs=int(os.environ.get("GNN_PSA", "2")), space="PSUM") as psA,
            tc.tile_pool(name="psB", bufs=int(os.environ.get("GNN_PSB", "2")), space="PSUM") as psB,
            tc.tile_pool(name="psT", bufs=2, space="PSUM") as psT,
        ):
            # --- persistent SBUF loads (idx prefix first: gathers wait on it) ---
            def load(shape, dt, src_ap, tag):
                t = pp.tile(shape, dt, tag=tag, name=tag)
                nc.sync.dma_start(out=t[:], in_=src_ap)
                return t

            idx_t = pp.tile([128, nw_total * 8], i16, tag="idx", name="idx")
            nc.sync.dma_start(
                out=idx_t[:, : nw_g0 * 8], in_=idx_in[:, : nw_g0 * 8]
            )
            nc.sync.dma_start(
                out=idx_t[:, nw_g0 * 8 :], in_=idx_in[:, nw_g0 * 8 :]
            )
            dslot_t = load([128, nw_total], f32, dslot_in[:, :], "dslot")
            iota_t = load([128, 128], f32, iota_in[:, :], "iota")
            ident_t = load([128, 128], bf16, ident_in[:, :], "ident")
            inv1_t = load([1, NPC_PAD], bf16, invdeg_in[:, :], "inv1")
            invdeg_t = pp.tile([128, NPC_PAD], bf16, tag="invdeg", name="invdeg")
            nc.gpsimd.partition_broadcast(invdeg_t[:], inv1_t[:])
            ws_t = [
                load([128, DIM], bf16, ws_in[l * DIM : (l + 1) * DIM, :], f"ws{l}")
                for l in range(N_LAYERS)
            ]
            wn_t = [
                load([128, DIM], bf16, wn_in[l * DIM : (l + 1) * DIM, :], f"wn{l}")
                for l in range(N_LAYERS)
            ]
            bias_t = [
                load([1, DIM], bf16, b_in[l : l + 1, :], f"bias{l}")
                for l in range(N_LAYERS)
            ]
            ones_t = pp.tile([1, 128], bf16, tag="ones", name="ones")
            nc.vector.memset(ones_t[:], 1.0)

            # h transposed (bf16) for the self path; ping-pong buffers
            hT = [
                load([DIM, NPC_PAD], bf16, xT_in[:, :], "hT0"),
                pp.tile([DIM, NPC_PAD], bf16, tag="hT1", name="hT1"),
            ]

            for l in range(N_LAYERS):
                srcs = [xA_in, xB_in] if l == 0 else [hgA[l - 1], hgB[l - 1]]
                hT_cur = hT[l % 2]
                hT_next = hT[(l + 1) % 2]
                for g, blocks in enumerate(groups):
                    # two gathers (one per src region) covering this group
                    msg_t = []
                    P_t = []
                    chunks = []
                    for r in range(2):
                        w0 = int(woff[blocks[0], r])
                        nw = int(sum(W[b, r] for b in blocks))
                        mt = msgp.tile(
                            [128, nw * DIM], bf16, tag=f"msg{r}", name=f"msg{r}"
                        )
                        msg_t.append((mt, w0))
                        for s0 in range(0, nw * BLK, CH):
                            chunks.append((r, mt, w0, s0, min(CH, nw * BLK - s0)))
                        # one-hot P for all nw windows in one DVE op:
                        # P[e, w, slot] = (iota[slot] == dslot[e, w])
                        Pw = pwp.tile(
                            [128, nw * BLK], bf16, tag=f"P{r}", name=f"P{r}"
                        )
                        nc.vector.tensor_tensor(
                            out=Pw[:].rearrange("p (w e) -> p w e", e=BLK),
                            in0=iota_t[:].unsqueeze(1).broadcast_to([128, nw, BLK]),
                            in1=dslot_t[:, w0 : w0 + nw]
                            .unsqueeze(2)
                            .broadcast_to([128, nw, BLK]),
                            op=mybir.AluOpType.is_equal,
                        )
                        P_t.append(Pw)
                    # issue region A/B chunks interleaved to keep queues smooth
                    chunks.sort(key=lambda t: t[3])
                    for r, mt, w0, s0, n in chunks:
                        nc.gpsimd.dma_gather(
                            out_ap=mt[:, s0 : s0 + n].rearrange(
                                "p (w e) -> p w e", e=DIM
                            ),
                            in_ap=srcs[r][:, :],
                            idxs_ap=idx_t[
                                :, w0 * 8 + s0 // 16 : w0 * 8 + (s0 + n) // 16
                            ],
                            num_idxs=n,
                            num_idxs_reg=n,
                            elem_size=DIM,
                            elem_step=DIM,
                            queue_num=qctr[0] % NQ,
                        )
                        qctr[0] += 1

                    for b in blocks:
                        pa = psA.tile([128, 128], f32, tag="agg", name="agg")
                        nwin_b = int(W[b, 0] + W[b, 1])
                        wi = 0
                        for r in range(2):
                            mt, w0 = msg_t[r]
                            Pw = P_t[r]
                            for k in range(int(W[b, r])):
                                wl = int(woff[b, r]) + k - w0  # window in segment
                                nc.tensor.matmul(
                                    pa[:],
                                    lhsT=mt[:, wl * DIM : (wl + 1) * DIM],
                                    rhs=Pw[:, wl * BLK : (wl + 1) * BLK],
                                    start=(wi == 0),
                                    stop=(wi == nwin_b - 1),
                                )
                                wi += 1
                        # aggT scaled by 1/deg (psum -> sbuf fused, bf16 out)
                        aggT = wkp.tile([128, 128], bf16, tag="aggT", name="aggT")
                        nc.vector.tensor_tensor(
                            out=aggT[:],
                            in0=pa[:],
                            in1=invdeg_t[:, b * BLK : (b + 1) * BLK],
                            op=mybir.AluOpType.mult,
                        )
                        # dense: out = aggT.T @ Wn + h.T.T @ Ws + 1 x bias
                        po = psB.tile([128, 128], f32, tag="out", name="outp")
                        nc.tensor.matmul(
                            po[:], lhsT=aggT[:],
                            rhs=wn_t[l][:],
                            start=True, stop=False,
                        )
                        nc.tensor.matmul(
                            po[:], lhsT=hT_cur[:, b * BLK : (b + 1) * BLK],
                            rhs=ws_t[l][:],
                            start=False, stop=False,
                        )
                        nc.tensor.matmul(
                            po[:], lhsT=ones_t[:],
                            rhs=bias_t[l][:],
                            start=False, stop=True,
                        )
                        rows = min(BLK, NPC - b * BLK)
                        if l < N_LAYERS - 1:
                            hbf = wkp.tile([128, 128], bf16, tag="hbf", name="hbf")
                            nc.scalar.activation(
                                hbf[:], po[:], mybir.ActivationFunctionType.Relu
                            )
                            nc.sync.dma_start(
                                out=hown[l][b * BLK : b * BLK + rows, :],
                                in_=hbf[:rows, :],
                            )
                            pt = psT.tile([128, 128], bf16, tag="tr", name="tr")
                            nc.tensor.transpose(
                                out=pt[:], in_=hbf[:], identity=ident_t[:]
                            )
                            nc.vector.tensor_copy(
                                out=hT_next[:, b * BLK : (b + 1) * BLK], in_=pt[:]
                            )
                            if chunk_ag and b == ABLK - 1:
                                # region A fully stored: fire its AllGather now
                                nc.gpsimd.collective_compute(
                                    "AllGather",
                                    mybir.AluOpType.bypass,
                                    replica_groups=rg,
                                    ins=[hown[l][0:AROWS, :]],
                                    outs=[hgA[l].ap()],
                                )
                        else:
                            ho = wkp.tile([128, 128], f32, tag="hnew", name="hnew")
                            nc.vector.tensor_copy(out=ho[:], in_=po[:])
                            nc.sync.dma_start(
                                out=out_ext[b * BLK : b * BLK + rows, :],
                                in_=ho[:rows, :],
                            )
                if l < N_LAYERS - 1:
                    if not chunk_ag:
                        nc.gpsimd.collective_compute(
                            "AllGather",
                            mybir.AluOpType.bypass,
                            replica_groups=rg,
                            ins=[hown[l][0:AROWS, :]],
                            outs=[hgA[l].ap()],
                        )
                    nc.gpsimd.collective_compute(
                        "AllGather",
                        mybir.AluOpType.bypass,
                        replica_groups=rg,
                        ins=[hown[l][AROWS:NPC, :]],
                        outs=[hgB[l].ap()],
                    )
    nc.compile()
    return nc


def kernel(x, src, dst, W_self, W_neigh, b):
    from concourse.bass_utils import run_bass_kernel_spmd

    x = np.asarray(x, np.float32)
    W_self = np.asarray(W_self, np.float32)
    W_neigh = np.asarray(W_neigh, np.float32)
    b = np.asarray(b, np.float32)
    src = np.asarray(src).astype(np.int64)
    dst = np.asarray(dst).astype(np.int64)

    perm = _pack_nodes(src, dst)
    psrc, pdst = perm[src], perm[dst]

    W, woff, nw_total, per_core = _preprocess(psrc, pdst)

    deg = np.bincount(pdst, minlength=N_NODES)
    invdeg = (1.0 / np.maximum(deg, 1)).astype(np.float32)

    nc = _build_program(W, woff, nw_total)

    inv_perm = np.empty(N_NODES, np.int64)
    inv_perm[perm] = np.arange(N_NODES)
    xp = x[inv_perm]  # xp[new_id] = x[orig]
    xbf = xp.astype(ml_dtypes.bfloat16)
    # region layout: A = per-core local rows [0, AROWS), B = rest
    x3 = xbf.reshape(CORES, NPC, DIM)
    xA = x3[:, :AROWS, :].reshape(NA, DIM).copy()
    xB = x3[:, AROWS:, :].reshape(NB, DIM).copy()
    iota = np.tile(np.arange(128, dtype=np.float32), (128, 1))
    ident = np.eye(128, dtype=ml_dtypes.bfloat16)
    ws_flat = W_self.reshape(N_LAYERS * DIM, DIM).astype(ml_dtypes.bfloat16)
    wn_flat = W_neigh.reshape(N_LAYERS * DIM, DIM).astype(ml_dtypes.bfloat16)
    b_bf = b.astype(ml_dtypes.bfloat16)

    in_maps = []
    for c in range(CORES):
        idx128, dslotT = per_core[c]
        inv_c = np.zeros((1, NPC_PAD), np.float32)
        inv_c[0, :NPC] = invdeg[c * NPC : (c + 1) * NPC]
        xT = np.zeros((DIM, NPC_PAD), ml_dtypes.bfloat16)
        xT[:, :NPC] = xp[c * NPC : (c + 1) * NPC].T
        in_maps.append(
            {
                "xA": xA,
                "xB": xB,
                "xT": xT,
                "idx": idx128,
                "dslot": dslotT,
                "invdeg": inv_c.astype(ml_dtypes.bfloat16),
                "Wself": ws_flat,
                "Wneigh": wn_flat,
                "bias": b_bf,
                "iota": iota,
                "ident": ident,
            }
        )

    trace = os.environ.get("GNN_TRACE", "0") == "1"
    if trace:
        try:
            import types

            import antenv

            if "antenv.axon_hooks" not in sys.modules:
                mod = types.ModuleType("antenv.axon_hooks")
                mod._HOOK = None

                def _set(h, _m=mod):
                    _m._HOOK = h

                def _get(_m=mod):
                    return _m._HOOK

                mod.set_axon_ntff_profile_hook = _set
                mod.get_axon_ntff_profile_hook = _get
                sys.modules["antenv.axon_hooks"] = mod
                antenv.axon_hooks = mod
            from trn_agent_boot.trn_boot import _ntff_profile_via_ctypes

            sys.modules["antenv.axon_hooks"].set_axon_ntff_profile_hook(
                _ntff_profile_via_ctypes("/opt/axon/libaxon_pjrt.so")
            )
        except Exception as e:  # profiling is best-effort
            print(f"ntff hook setup failed: {e}")
            trace = False
    res = run_bass_kernel_spmd(
        nc, in_maps, core_ids=list(range(CORES)), trace=trace
    )
    LAST_EXEC_NS[0] = res.exec_time_ns
    LAST_PROFILE[0] = res.profile_json

    out_p = np.concatenate([res.results[c]["out"] for c in range(CORES)], axis=0)
    return out_p[perm].astype(np.float32)


# revision 14
# speedup vs baseline: 1.1244x; 1.1244x over previous
"""3-layer GraphSAGE (mean agg) on 8 trn2 NeuronCores.

Sharding: nodes data-parallel (6250/core), weights replicated. A global node
relabeling (region-preserving 2-D bin packing on per-node in-degree from each
source region) assigns nodes to (core, block) so per-(block,region) edge
counts sit just under multiples of 128, cutting gather padding to ~2%.

Per core: edges with dst in its node range, grouped (group-major) by
(dst-block-group, src-region), padded to core-invariant per-(block,region)
window counts so one SPMD program works for all cores. Neighbor features
fetched by dma_gather (bf16 256B rows) spread round-robin over 4 SWDGE
queues, aggregated per 128-edge window by one-hot matmuls accumulating in
PSUM (one-hot P built in bulk per gather segment via broadcast-AP is_equal),
then fused dense layer in bf16.

Source nodes split in two regions by local row (A: rows 0..3071 per core,
B: rest) so the inter-layer AllGather runs as two chunks; chunk A can fire
mid-layer (after block 23) and overlap the remaining compute. Gathers
address hgA [8*3072, D] / hgB [8*3178, D] with int16 offsets.
"""

import os
import sys

sys.path.insert(0, "/opt/trn_rl_repo")

import numpy as np
import ml_dtypes

N_NODES = 50000
N_EDGES = 800000
DIM = 128
N_LAYERS = 3
CORES = 8
NPC = N_NODES // CORES          # 6250 nodes per core
BLK = 128
NBLK = (NPC + BLK - 1) // BLK   # 49 blocks (last has 106 valid rows)
NPC_PAD = NBLK * BLK            # 6272
ABLK = 24                       # blocks in region A (per core)
AROWS = ABLK * BLK              # 3072
BROWS = NPC - AROWS             # 3178
NA = CORES * AROWS              # 24576 rows in hgA (int16-safe < 32768)
NB = CORES * BROWS              # 25424 rows in hgB
GB = int(os.environ.get("GNN_GB", "3"))   # blocks per gather group
NQ = 4                          # SWDGE queues (ucode max)
CH = 1024                       # idx per gather inst (= ring capacity/queue)

LAST_EXEC_NS = [None]
LAST_PROFILE = [None]


def _pack_nodes(src, dst):
    """Region-preserving node relabeling: 2-D best-fit-decreasing packing of
    nodes into (core, block) bins so per-(block, src-region) in-degree sums
    sit just under multiples of 128. Returns perm (old id -> new id)."""
    l_s = src % NPC
    isA_src = l_s < AROWS
    dA = np.bincount(dst[isA_src], minlength=N_NODES).astype(np.float64)
    dB = np.bincount(dst[~isA_src], minlength=N_NODES).astype(np.float64)
    isA_node = (np.arange(N_NODES) % NPC) < AROWS

    def mkcaps(npos, sumA, sumB):
        baseA, extraA = divmod(sumA, npos)
        capA = np.full(npos, baseA)
        capA[:extraA] += 1
        baseB, extraB = divmod(sumB, npos)
        capB = np.full(npos, baseB)
        capB[npos - extraB :] += 1
        return capA * 128.0, capB * 128.0

    def pack(nodes, npos, cards, slackA, slackB):
        sumA = int(np.ceil(dA[nodes].sum() / CORES / 128)) + slackA
        sumB = int(np.ceil(dB[nodes].sum() / CORES / 128)) + slackB
        capA, capB = mkcaps(npos, sumA, sumB)
        nbins = CORES * npos
        bcapA = np.tile(capA, CORES)
        bcapB = np.tile(capB, CORES)
        bcard = np.tile(cards, CORES)
        da, db = dA[nodes], dB[nodes]
        order = np.argsort(-(da + db))
        sA = np.zeros(nbins)
        sB = np.zeros(nbins)
        cnt = np.zeros(nbins, np.int64)
        binof = np.empty(len(nodes), np.int64)
        for i in order:
            fa, fb = sA + da[i], sB + db[i]
            feas = (fa <= bcapA) & (fb <= bcapB) & (cnt < bcard)
            if feas.any():
                score = np.where(
                    feas,
                    np.maximum(fa / bcapA, fb / bcapB) - 0.3 * cnt / bcard,
                    np.inf,
                )
            else:
                score = np.where(
                    cnt < bcard, np.maximum(fa / bcapA, fb / bcapB), np.inf
                )
            j = int(np.argmin(score))
            binof[i] = j
            sA[j] += da[i]
            sB[j] += db[i]
            cnt[j] += 1
        assert (cnt == bcard).all()
        return binof

    perm = np.empty(N_NODES, np.int64)
    nodesA = np.nonzero(isA_node)[0]
    binA = pack(nodesA, ABLK, np.full(ABLK, 128), 3, 2)
    # stable order within bin
    order = np.lexsort((nodesA, binA))
    slot = np.arange(len(nodesA)) - np.searchsorted(binA[order], binA[order])
    c, p = binA[order] // ABLK, binA[order] % ABLK
    perm[nodesA[order]] = c * NPC + p * 128 + slot

    nodesB = np.nonzero(~isA_node)[0]
    nposB = NBLK - ABLK  # 25
    cardsB = np.array([128] * (nposB - 1) + [NPC - AROWS - (nposB - 1) * 128])
    binB = pack(nodesB, nposB, cardsB, 3, 2)
    order = np.lexsort((nodesB, binB))
    slot = np.arange(len(nodesB)) - np.searchsorted(binB[order], binB[order])
    c, p = binB[order] // nposB, binB[order] % nposB
    perm[nodesB[order]] = c * NPC + AROWS + p * 128 + slot
    return perm


def _src_region(src):
    """Map (permuted) src node id -> (region, offset-in-region-tensor)."""
    c = src // NPC
    l = src % NPC
    isB = l >= AROWS
    off = np.where(isB, c * BROWS + (l - AROWS), c * AROWS + l)
    return isB.astype(np.int64), off


def _preprocess(src, dst):
    """Host-side graph preprocessing on permuted ids. Returns per-core
    index/dslot arrays plus the shared (core-invariant) window schedule.
    Window stream order is group-major: for g: for r: for b in group."""
    src = np.asarray(src).astype(np.int64)
    dst = np.asarray(dst).astype(np.int64)

    owner = dst // NPC
    run, soff = _src_region(src)
    dloc = dst - owner * NPC
    blk = dloc // BLK
    grp = blk // GB

    counts = np.zeros((CORES, NBLK, 2), np.int64)
    np.add.at(counts, (owner, blk, run), 1)
    W = np.maximum(1, -(-counts.max(axis=0) // BLK))  # [NBLK, 2]

    groups = [list(range(s, min(s + GB, NBLK))) for s in range(0, NBLK, GB)]
    # group-major window offsets
    woff = np.zeros((NBLK, 2), np.int64)
    w = 0
    for g, blocks in enumerate(groups):
        for r in range(2):
            for b in blocks:
                woff[b, r] = w
                w += int(W[b, r])
    nw_total = w

    per_core = []
    for c in range(CORES):
        sel = owner == c
        es, eb, er, ed, eg = soff[sel], blk[sel], run[sel], dloc[sel], grp[sel]
        order = np.lexsort((eb, er, eg))
        es, eb, er, ed = es[order], eb[order], er[order], ed[order]

        idx_out = np.zeros(nw_total * BLK, np.int16)
        dslot_out = np.full(nw_total * BLK, 255.0, np.float32)
        epos = 0
        for g, blocks in enumerate(groups):
            for r in range(2):
                for b in blocks:
                    cnt = int(counts[c, b, r])
                    e0, e1 = epos, epos + cnt
                    o0 = int(woff[b, r]) * BLK
                    idx_out[o0 : o0 + cnt] = es[e0:e1].astype(np.int16)
                    dslot_out[o0 : o0 + cnt] = (ed[e0:e1] - b * BLK).astype(
                        np.float32
                    )
                    # pads: idx 0 (valid row, gathered but zeroed by P)
                    epos = e1
        assert epos == len(es)

        # wrap indices in 16 partitions, replicate to 128 (one copy / Q7 core)
        wrapped = idx_out.reshape(-1, 16).T.copy()        # [16, nw_total*8]
        idx128 = np.tile(wrapped, (8, 1))                 # [128, nw_total*8]
        # dslotT: [128, nw_total]; column w = dslots of window w's 128 edges
        dslotT = dslot_out.reshape(nw_total, BLK).T.copy()
        per_core.append((idx128, dslotT))

    return W, woff, nw_total, per_core


def _build_program(W, woff, nw_total):
    import concourse.bass as bass
    import concourse.mybir as mybir
    import concourse.tile as tile
    from concourse import bacc

    f32 = mybir.dt.float32
    bf16 = mybir.dt.bfloat16
    i16 = mybir.dt.int16

    nc = bacc.Bacc(
        "TRN2",
        target_bir_lowering=False,
        num_devices=CORES,
        num_swdge_queues=NQ,
        dynamic_dma_scratch_size=int(os.environ.get("GNN_SCRATCH", "32768")),
    )

    # I/O (xA/xB: layer-0 gather sources in region layout)
    xA_in = nc.declare_dram_parameter("xA", [NA, DIM], bf16, isOutput=False)
    xB_in = nc.declare_dram_parameter("xB", [NB, DIM], bf16, isOutput=False)
    xT_in = nc.declare_dram_parameter("xT", [DIM, NPC_PAD], bf16, isOutput=False)
    idx_in = nc.declare_dram_parameter("idx", [128, nw_total * 8], i16, isOutput=False)
    dslot_in = nc.declare_dram_parameter("dslot", [128, nw_total], f32, isOutput=False)
    invdeg_in = nc.declare_dram_parameter("invdeg", [1, NPC_PAD], bf16, isOutput=False)
    ws_in = nc.declare_dram_parameter("Wself", [N_LAYERS * DIM, DIM], bf16, isOutput=False)
    wn_in = nc.declare_dram_parameter("Wneigh", [N_LAYERS * DIM, DIM], bf16, isOutput=False)
    b_in = nc.declare_dram_parameter("bias", [N_LAYERS, DIM], bf16, isOutput=False)
    iota_in = nc.declare_dram_parameter("iota", [128, 128], f32, isOutput=False)
    ident_in = nc.declare_dram_parameter("ident", [128, 128], bf16, isOutput=False)
    out_ext = nc.declare_dram_parameter("out", [NPC, DIM], f32, isOutput=True)

    # internal DRAM for collectives (two chunks per boundary)
    hown = [nc.dram_tensor(f"hown{l}", [NPC, DIM], bf16) for l in range(2)]
    hgA = [
        nc.dram_tensor(f"hgA{l}", [NA, DIM], bf16, addr_space="Shared")
        for l in range(2)
    ]
    hgB = [
        nc.dram_tensor(f"hgB{l}", [NB, DIM], bf16, addr_space="Shared")
        for l in range(2)
    ]
    rg = [list(range(CORES))]
    chunk_ag = os.environ.get("GNN_CHUNK_AG", "0") == "1"

    groups = [list(range(s, min(s + GB, NBLK))) for s in range(0, NBLK, GB)]
    # idx prefix: windows of group 0 (both regions) for the early small load
    nw_g0 = int(sum(int(W[b, r]) for r in range(2) for b in groups[0]))

    qctr = [0]  # SWDGE queue round-robin across all gathers

    with tile.TileContext(nc) as tc:
        with (
            tc.tile_pool(name="persist", bufs=1) as pp,
            tc.tile_pool(name="msg", bufs=int(os.environ.get("GNN_MSGBUFS", "4"))) as msgp,
            tc.tile_pool(name="pwin", bufs=int(os.environ.get("GNN_PBUFS", "3"))) as pwp,
            tc.tile_pool(name="work", bufs=4) as wkp,
            tc.tile_pool(name="psA", bufs=int(os.environ.get("GNN_PSA", "2")), space="PSUM") as psA,
            tc.tile_pool(name="psB", bufs=int(os.environ.get("GNN_PSB", "2")), space="PSUM") as psB,
            tc.tile_pool(name="psT", bufs=2, space="PSUM") as psT,
        ):
            # --- persistent SBUF loads (idx prefix first: gathers wait on it) ---
            def load(shape, dt, src_ap, tag):
                t = pp.tile(shape, dt, tag=tag, name=tag)
                nc.sync.dma_start(out=t[:], in_=src_ap)
                return t

            idx_t = pp.tile([128, nw_total * 8], i16, tag="idx", name="idx")
            nc.sync.dma_start(
                out=idx_t[:, : nw_g0 * 8], in_=idx_in[:, : nw_g0 * 8]
            )
            nc.sync.dma_start(
                out=idx_t[:, nw_g0 * 8 :], in_=idx_in[:, nw_g0 * 8 :]
            )
            dslot_t = load([128, nw_total], f32, dslot_in[:, :], "dslot")
            iota_t = load([128, 128], f32, iota_in[:, :], "iota")
            ident_t = load([128, 128], bf16, ident_in[:, :], "ident")
            inv1_t = load([1, NPC_PAD], bf16, invdeg_in[:, :], "inv1")
            invdeg_t = pp.tile([128, NPC_PAD], bf16, tag="invdeg", name="invdeg")
            nc.gpsimd.partition_broadcast(invdeg_t[:], inv1_t[:])
            ws_t = [
                load([128, DIM], bf16, ws_in[l * DIM : (l + 1) * DIM, :], f"ws{l}")
                for l in range(N_LAYERS)
            ]
            wn_t = [
                load([128, DIM], bf16, wn_in[l * DIM : (l + 1) * DIM, :], f"wn{l}")
                for l in range(N_LAYERS)
            ]
            bias_t = [
                load([1, DIM], bf16, b_in[l : l + 1, :], f"bias{l}")
                for l in range(N_LAYERS)
            ]
            ones_t = pp.tile([1, 128], bf16, tag="ones", name="ones")
            nc.vector.memset(ones_t[:], 1.0)

            # h transposed (bf16) for the self path; ping-pong buffers
            hT = [
                load([DIM, NPC_PAD], bf16, xT_in[:, :], "hT0"),
                pp.tile([DIM, NPC_PAD], bf16, tag="hT1", name="hT1"),
            ]

            for l in range(N_LAYERS):
                srcs = [xA_in, xB_in] if l == 0 else [hgA[l - 1], hgB[l - 1]]
                hT_cur = hT[l % 2]
                hT_next = hT[(l + 1) % 2]
                for g, blocks in enumerate(groups):
                    # two gathers (one per src region) covering this group
                    msg_t = []
                    P_t = []
                    chunks = []
                    for r in range(2):
                        w0 = int(woff[blocks[0], r])
                        nw = int(sum(W[b, r] for b in blocks))
                        mt = msgp.tile(
                            [128, nw * DIM], bf16, tag=f"msg{r}", name=f"msg{r}"
                        )
                        msg_t.append((mt, w0))
                        for s0 in range(0, nw * BLK, CH):
                            chunks.append((r, mt, w0, s0, min(CH, nw * BLK - s0)))
                        # one-hot P for all nw windows in one DVE op:
                        # P[e, w, slot] = (iota[slot] == dslot[e, w])
                        Pw = pwp.tile(
                            [128, nw * BLK], bf16, tag=f"P{r}", name=f"P{r}"
                        )
                        nc.vector.tensor_tensor(
                            out=Pw[:].rearrange("p (w e) -> p w e", e=BLK),
                            in0=iota_t[:].unsqueeze(1).broadcast_to([128, nw, BLK]),
                            in1=dslot_t[:, w0 : w0 + nw]
                            .unsqueeze(2)
                            .broadcast_to([128, nw, BLK]),
                            op=mybir.AluOpType.is_equal,
                        )
                        P_t.append(Pw)
                    for r, mt, w0, s0, n in chunks:
                        nc.gpsimd.dma_gather(
                            out_ap=mt[:, s0 : s0 + n].rearrange(
                                "p (w e) -> p w e", e=DIM
                            ),
                            in_ap=srcs[r][:, :],
                            idxs_ap=idx_t[
                                :, w0 * 8 + s0 // 16 : w0 * 8 + (s0 + n) // 16
                            ],
                            num_idxs=n,
                            num_idxs_reg=n,
                            elem_size=DIM,
                            elem_step=DIM,
                            queue_num=qctr[0] % NQ,
                        )
                        qctr[0] += 1

                    for b in blocks:
                        pa = psA.tile([128, 128], f32, tag="agg", name="agg")
                        nwin_b = int(W[b, 0] + W[b, 1])
                        wi = 0
                        for r in range(2):
                            mt, w0 = msg_t[r]
                            Pw = P_t[r]
                            for k in range(int(W[b, r])):
                                wl = int(woff[b, r]) + k - w0  # window in segment
                                nc.tensor.matmul(
                                    pa[:],
                                    lhsT=mt[:, wl * DIM : (wl + 1) * DIM],
                                    rhs=Pw[:, wl * BLK : (wl + 1) * BLK],
                                    start=(wi == 0),
                                    stop=(wi == nwin_b - 1),
                                )
                                wi += 1
                        # aggT scaled by 1/deg (psum -> sbuf fused, bf16 out)
                        aggT = wkp.tile([128, 128], bf16, tag="aggT", name="aggT")
                        nc.vector.tensor_tensor(
                            out=aggT[:],
                            in0=pa[:],
                            in1=invdeg_t[:, b * BLK : (b + 1) * BLK],
                            op=mybir.AluOpType.mult,
                        )
                        # dense: out = aggT.T @ Wn + h.T.T @ Ws + 1 x bias
                        po = psB.tile([128, 128], f32, tag="out", name="outp")
                        nc.tensor.matmul(
                            po[:], lhsT=aggT[:],
                            rhs=wn_t[l][:],
                            start=True, stop=False,
                        )
                        nc.tensor.matmul(
                            po[:], lhsT=hT_cur[:, b * BLK : (b + 1) * BLK],
                            rhs=ws_t[l][:],
                            start=False, stop=False,
                        )
                        nc.tensor.matmul(
                            po[:], lhsT=ones_t[:],
                            rhs=bias_t[l][:],
                            start=False, stop=True,
                        )
                        rows = min(BLK, NPC - b * BLK)
                        if l < N_LAYERS - 1:
                            hbf = wkp.tile([128, 128], bf16, tag="hbf", name="hbf")
                            nc.scalar.activation(
                                hbf[:], po[:], mybir.ActivationFunctionType.Relu
                            )
                            nc.sync.dma_start(
                                out=hown[l][b * BLK : b * BLK + rows, :],
                                in_=hbf[:rows, :],
                            )
                            pt = psT.tile([128, 128], bf16, tag="tr", name="tr")
                            nc.tensor.transpose(
                                out=pt[:], in_=hbf[:], identity=ident_t[:]
                            )
                            nc.vector.tensor_copy(
                                out=hT_next[:, b * BLK : (b + 1) * BLK], in_=pt[:]
                            )
                            if chunk_ag and b == ABLK - 1:
                                # region A fully stored: fire its AllGather now
                                nc.gpsimd.collective_compute(
                                    "AllGather",
                                    mybir.AluOpType.bypass,
                                    replica_groups=rg,
                                    ins=[hown[l][0:AROWS, :]],
                                    outs=[hgA[l].ap()],
                                )
                        else:
                            ho = wkp.tile([128, 128], f32, tag="hnew", name="hnew")
                            nc.vector.tensor_copy(out=ho[:], in_=po[:])
                            nc.sync.dma_start(
                                out=out_ext[b * BLK : b * BLK + rows, :],
                                in_=ho[:rows, :],
                            )
                if l < N_LAYERS - 1:
                    if not chunk_ag:
                        nc.gpsimd.collective_compute(
                            "AllGather",
                            mybir.AluOpType.bypass,
                            replica_groups=rg,
                            ins=[hown[l][0:AROWS, :]],
                            outs=[hgA[l].ap()],
                        )
                    nc.gpsimd.collective_compute(
                        "AllGather",
                        mybir.AluOpType.bypass,
                        replica_groups=rg,
                        ins=[hown[l][AROWS:NPC, :]],
                        outs=[hgB[l].ap()],
                    )
    nc.compile()
    return nc


def kernel(x, src, dst, W_self, W_neigh, b):
    from concourse.bass_utils import run_bass_kernel_spmd

    x = np.asarray(x, np.float32)
    W_self = np.asarray(W_self, np.float32)
    W_neigh = np.asarray(W_neigh, np.float32)
    b = np.asarray(b, np.float32)
    src = np.asarray(src).astype(np.int64)
    dst = np.asarray(dst).astype(np.int64)

    perm = _pack_nodes(src, dst)
    psrc, pdst = perm[src], perm[dst]

    W, woff, nw_total, per_core = _preprocess(psrc, pdst)

    deg = np.bincount(pdst, minlength=N_NODES)
    invdeg = (1.0 / np.maximum(deg, 1)).astype(np.float32)

    nc = _build_program(W, woff, nw_total)

    inv_perm = np.empty(N_NODES, np.int64)
    inv_perm[perm] = np.arange(N_NODES)
    xp = x[inv_perm]  # xp[new_id] = x[orig]
    xbf = xp.astype(ml_dtypes.bfloat16)
    # region layout: A = per-core local rows [0, AROWS), B = rest
    x3 = xbf.reshape(CORES, NPC, DIM)
    xA = x3[:, :AROWS, :].reshape(NA, DIM).copy()
    xB = x3[:, AROWS:, :].reshape(NB, DIM).copy()
    iota = np.tile(np.arange(128, dtype=np.float32), (128, 1))
    ident = np.eye(128, dtype=ml_dtypes.bfloat16)
    ws_flat = W_self.reshape(N_LAYERS * DIM, DIM).astype(ml_dtypes.bfloat16)
    wn_flat = W_neigh.reshape(N_LAYERS * DIM, DIM).astype(ml_dtypes.bfloat16)
    b_bf = b.astype(ml_dtypes.bfloat16)

    in_maps = []
    for c in range(CORES):
        idx128, dslotT = per_core[c]
        inv_c = np.zeros((1, NPC_PAD), np.float32)
        inv_c[0, :NPC] = invdeg[c * NPC : (c + 1) * NPC]
        xT = np.zeros((DIM, NPC_PAD), ml_dtypes.bfloat16)
        xT[:, :NPC] = xp[c * NPC : (c + 1) * NPC].T
        in_maps.append(
            {
                "xA": xA,
                "xB": xB,
                "xT": xT,
                "idx": idx128,
                "dslot": dslotT,
                "invdeg": inv_c.astype(ml_dtypes.bfloat16),
                "Wself": ws_flat,
                "Wneigh": wn_flat,
                "bias": b_bf,
                "iota": iota,
                "ident": ident,
            }
        )

    trace = os.environ.get("GNN_TRACE", "0") == "1"
    if trace:
        try:
            import types

            import antenv

            if "antenv.axon_hooks" not in sys.modules:
                mod = types.ModuleType("antenv.axon_hooks")
                mod._HOOK = None

                def _set(h, _m=mod):
                    _m._HOOK = h

                def _get(_m=mod):
                    return _m._HOOK

                mod.set_axon_ntff_profile_hook = _set
                mod.get_axon_ntff_profile_hook = _get
                sys.modules["antenv.axon_hooks"] = mod
                antenv.axon_hooks = mod
            from trn_agent_boot.trn_boot import _ntff_profile_via_ctypes

            sys.modules["antenv.axon_hooks"].set_axon_ntff_profile_hook(
                _ntff_profile_via_ctypes("/opt/axon/libaxon_pjrt.so")
            )
        except Exception as e:  # profiling is best-effort
            print(f"ntff hook setup failed: {e}")
            trace = False
    res = run_bass_kernel_spmd(
        nc, in_maps, core_ids=list(range(CORES)), trace=trace
    )
    LAST_EXEC_NS[0] = res.exec_time_ns
    LAST_PROFILE[0] = res.profile_json

    out_p = np.concatenate([res.results[c]["out"] for c in range(CORES)], axis=0)
    return out_p[perm].astype(np.float32)


# revision 15
# speedup vs baseline: 1.2904x; 1.1477x over previous
"""3-layer GraphSAGE (mean agg) on 8 trn2 NeuronCores.

Sharding: nodes data-parallel (6250/core), weights replicated. A global node
relabeling (region-preserving 2-D bin packing on per-node in-degree from each
source region) assigns nodes to (core, block) so per-(block,region) edge
counts sit just under multiples of 128, cutting gather padding to ~2%.

Per core: edges with dst in its node range, grouped (group-major) by
(dst-block-group, src-region), padded to core-invariant per-(block,region)
window counts so one SPMD program works for all cores. Neighbor features
fetched by dma_gather (bf16 256B rows) spread round-robin over 4 SWDGE
queues, aggregated per 128-edge window by one-hot matmuls accumulating in
PSUM (one-hot P built in bulk per gather segment via broadcast-AP is_equal),
then fused dense layer in bf16.

Source nodes split in two regions by local row (A: rows 0..3071 per core,
B: rest) so the inter-layer AllGather runs as two chunks; chunk A can fire
mid-layer (after block 23) and overlap the remaining compute. Gathers
address hgA [8*3072, D] / hgB [8*3178, D] with int16 offsets.
"""

import os
import sys

sys.path.insert(0, "/opt/trn_rl_repo")

import numpy as np
import ml_dtypes

N_NODES = 50000
N_EDGES = 800000
DIM = 128
N_LAYERS = 3
CORES = 8
NPC = N_NODES // CORES          # 6250 nodes per core
BLK = 128
NBLK = (NPC + BLK - 1) // BLK   # 49 blocks (last has 106 valid rows)
NPC_PAD = NBLK * BLK            # 6272
ABLK = 24                       # blocks in region A (per core)
AROWS = ABLK * BLK              # 3072
BROWS = NPC - AROWS             # 3178
NA = CORES * AROWS              # 24576 rows in hgA (int16-safe < 32768)
NB = CORES * BROWS              # 25424 rows in hgB
GB = int(os.environ.get("GNN_GB", "3"))   # blocks per gather group
NQ = 4                          # SWDGE queues (ucode max)
CH = 1024                       # idx per gather inst (= ring capacity/queue)

LAST_EXEC_NS = [None]
LAST_PROFILE = [None]


def _pack_nodes(src, dst):
    """Region-preserving node relabeling: 2-D best-fit-decreasing packing of
    nodes into (core, block) bins so per-(block, src-region) in-degree sums
    sit just under multiples of 128. Returns perm (old id -> new id)."""
    l_s = src % NPC
    isA_src = l_s < AROWS
    dA = np.bincount(dst[isA_src], minlength=N_NODES).astype(np.float64)
    dB = np.bincount(dst[~isA_src], minlength=N_NODES).astype(np.float64)
    isA_node = (np.arange(N_NODES) % NPC) < AROWS

    def mkcaps(npos, sumA, sumB):
        baseA, extraA = divmod(sumA, npos)
        capA = np.full(npos, baseA)
        capA[:extraA] += 1
        baseB, extraB = divmod(sumB, npos)
        capB = np.full(npos, baseB)
        capB[npos - extraB :] += 1
        return capA * 128.0, capB * 128.0

    def pack(nodes, npos, cards, slackA, slackB):
        sumA = int(np.ceil(dA[nodes].sum() / CORES / 128)) + slackA
        sumB = int(np.ceil(dB[nodes].sum() / CORES / 128)) + slackB
        capA, capB = mkcaps(npos, sumA, sumB)
        nbins = CORES * npos
        bcapA = np.tile(capA, CORES)
        bcapB = np.tile(capB, CORES)
        bcard = np.tile(cards, CORES)
        da, db = dA[nodes], dB[nodes]
        order = np.argsort(-(da + db))
        sA = np.zeros(nbins)
        sB = np.zeros(nbins)
        cnt = np.zeros(nbins, np.int64)
        binof = np.empty(len(nodes), np.int64)
        for i in order:
            fa, fb = sA + da[i], sB + db[i]
            feas = (fa <= bcapA) & (fb <= bcapB) & (cnt < bcard)
            if feas.any():
                score = np.where(
                    feas,
                    np.maximum(fa / bcapA, fb / bcapB) - 0.3 * cnt / bcard,
                    np.inf,
                )
            else:
                score = np.where(
                    cnt < bcard, np.maximum(fa / bcapA, fb / bcapB), np.inf
                )
            j = int(np.argmin(score))
            binof[i] = j
            sA[j] += da[i]
            sB[j] += db[i]
            cnt[j] += 1
        assert (cnt == bcard).all()
        return binof

    perm = np.empty(N_NODES, np.int64)
    nodesA = np.nonzero(isA_node)[0]
    binA = pack(nodesA, ABLK, np.full(ABLK, 128), 3, 2)
    # stable order within bin
    order = np.lexsort((nodesA, binA))
    slot = np.arange(len(nodesA)) - np.searchsorted(binA[order], binA[order])
    c, p = binA[order] // ABLK, binA[order] % ABLK
    perm[nodesA[order]] = c * NPC + p * 128 + slot

    nodesB = np.nonzero(~isA_node)[0]
    nposB = NBLK - ABLK  # 25
    cardsB = np.array([128] * (nposB - 1) + [NPC - AROWS - (nposB - 1) * 128])
    binB = pack(nodesB, nposB, cardsB, 3, 2)
    order = np.lexsort((nodesB, binB))
    slot = np.arange(len(nodesB)) - np.searchsorted(binB[order], binB[order])
    c, p = binB[order] // nposB, binB[order] % nposB
    perm[nodesB[order]] = c * NPC + AROWS + p * 128 + slot
    return perm


def _src_region(src):
    """Map (permuted) src node id -> (region, offset-in-region-tensor)."""
    c = src // NPC
    l = src % NPC
    isB = l >= AROWS
    off = np.where(isB, c * BROWS + (l - AROWS), c * AROWS + l)
    return isB.astype(np.int64), off


def _preprocess(src, dst):
    """Host-side graph preprocessing on permuted ids. Returns per-core
    index/dslot arrays plus the shared (core-invariant) window schedule.
    Window stream order is group-major: for g: for r: for b in group."""
    src = np.asarray(src).astype(np.int64)
    dst = np.asarray(dst).astype(np.int64)

    owner = dst // NPC
    run, soff = _src_region(src)
    dloc = dst - owner * NPC
    blk = dloc // BLK
    grp = blk // GB

    counts = np.zeros((CORES, NBLK, 2), np.int64)
    np.add.at(counts, (owner, blk, run), 1)
    W = np.maximum(1, -(-counts.max(axis=0) // BLK))  # [NBLK, 2]

    groups = [list(range(s, min(s + GB, NBLK))) for s in range(0, NBLK, GB)]
    # group-major window offsets
    woff = np.zeros((NBLK, 2), np.int64)
    w = 0
    for g, blocks in enumerate(groups):
        for r in range(2):
            for b in blocks:
                woff[b, r] = w
                w += int(W[b, r])
    nw_total = w

    per_core = []
    for c in range(CORES):
        sel = owner == c
        es, eb, er, ed, eg = soff[sel], blk[sel], run[sel], dloc[sel], grp[sel]
        order = np.lexsort((eb, er, eg))
        es, eb, er, ed = es[order], eb[order], er[order], ed[order]

        idx_out = np.zeros(nw_total * BLK, np.int16)
        dslot_out = np.full(nw_total * BLK, 255.0, np.float32)
        epos = 0
        for g, blocks in enumerate(groups):
            for r in range(2):
                for b in blocks:
                    cnt = int(counts[c, b, r])
                    e0, e1 = epos, epos + cnt
                    o0 = int(woff[b, r]) * BLK
                    idx_out[o0 : o0 + cnt] = es[e0:e1].astype(np.int16)
                    dslot_out[o0 : o0 + cnt] = (ed[e0:e1] - b * BLK).astype(
                        np.float32
                    )
                    # pads: idx 0 (valid row, gathered but zeroed by P)
                    epos = e1
        assert epos == len(es)

        # wrap indices in 16 partitions, replicate to 128 (one copy / Q7 core)
        wrapped = idx_out.reshape(-1, 16).T.copy()        # [16, nw_total*8]
        idx128 = np.tile(wrapped, (8, 1))                 # [128, nw_total*8]
        # dslotT: [128, nw_total]; column w = dslots of window w's 128 edges
        dslotT = dslot_out.reshape(nw_total, BLK).T.copy()
        per_core.append((idx128, dslotT))

    return W, woff, nw_total, per_core


def _build_program(W, woff, nw_total):
    import concourse.bass as bass
    import concourse.mybir as mybir
    import concourse.tile as tile
    from concourse import bacc

    f32 = mybir.dt.float32
    bf16 = mybir.dt.bfloat16
    i16 = mybir.dt.int16

    nc = bacc.Bacc(
        "TRN2",
        target_bir_lowering=False,
        num_devices=CORES,
        num_swdge_queues=NQ,
        dynamic_dma_scratch_size=int(os.environ.get("GNN_SCRATCH", "32768")),
    )

    # I/O (xA/xB: layer-0 gather sources in region layout)
    xA_in = nc.declare_dram_parameter("xA", [NA, DIM], bf16, isOutput=False)
    xB_in = nc.declare_dram_parameter("xB", [NB, DIM], bf16, isOutput=False)
    xT_in = nc.declare_dram_parameter("xT", [DIM, NPC_PAD], bf16, isOutput=False)
    idx_in = nc.declare_dram_parameter("idx", [128, nw_total * 8], i16, isOutput=False)
    dslot_in = nc.declare_dram_parameter("dslot", [128, nw_total], f32, isOutput=False)
    invdeg_in = nc.declare_dram_parameter("invdeg", [1, NPC_PAD], bf16, isOutput=False)
    ws_in = nc.declare_dram_parameter("Wself", [N_LAYERS * DIM, DIM], bf16, isOutput=False)
    wn_in = nc.declare_dram_parameter("Wneigh", [N_LAYERS * DIM, DIM], bf16, isOutput=False)
    b_in = nc.declare_dram_parameter("bias", [N_LAYERS, DIM], bf16, isOutput=False)
    iota_in = nc.declare_dram_parameter("iota", [128, 128], f32, isOutput=False)
    ident_in = nc.declare_dram_parameter("ident", [128, 128], bf16, isOutput=False)
    out_ext = nc.declare_dram_parameter("out", [NPC, DIM], f32, isOutput=True)

    # internal DRAM for collectives (two chunks per boundary)
    hown = [nc.dram_tensor(f"hown{l}", [NPC, DIM], bf16) for l in range(2)]
    hgA = [
        nc.dram_tensor(f"hgA{l}", [NA, DIM], bf16, addr_space="Shared")
        for l in range(2)
    ]
    hgB = [
        nc.dram_tensor(f"hgB{l}", [NB, DIM], bf16, addr_space="Shared")
        for l in range(2)
    ]
    rg = [list(range(CORES))]
    chunk_ag = os.environ.get("GNN_CHUNK_AG", "0") == "1"

    groups = [list(range(s, min(s + GB, NBLK))) for s in range(0, NBLK, GB)]
    # idx prefix: windows of group 0 (both regions) for the early small load
    nw_g0 = int(sum(int(W[b, r]) for r in range(2) for b in groups[0]))

    qctr = [0]  # SWDGE queue round-robin across all gathers

    with tile.TileContext(nc) as tc:
        with (
            tc.tile_pool(name="persist", bufs=1) as pp,
            tc.tile_pool(name="msg", bufs=int(os.environ.get("GNN_MSGBUFS", "4"))) as msgp,
            tc.tile_pool(name="pwin", bufs=int(os.environ.get("GNN_PBUFS", "3"))) as pwp,
            tc.tile_pool(name="work", bufs=4) as wkp,
            tc.tile_pool(name="psA", bufs=int(os.environ.get("GNN_PSA", "2")), space="PSUM") as psA,
            tc.tile_pool(name="psB", bufs=int(os.environ.get("GNN_PSB", "2")), space="PSUM") as psB,
            tc.tile_pool(name="psT", bufs=2, space="PSUM") as psT,
        ):
            # --- persistent SBUF loads (idx prefix first: gathers wait on it) ---
            def load(shape, dt, src_ap, tag):
                t = pp.tile(shape, dt, tag=tag, name=tag)
                nc.sync.dma_start(out=t[:], in_=src_ap)
                return t

            idx_t = pp.tile([128, nw_total * 8], i16, tag="idx", name="idx")
            nc.sync.dma_start(
                out=idx_t[:, : nw_g0 * 8], in_=idx_in[:, : nw_g0 * 8]
            )
            nc.sync.dma_start(
                out=idx_t[:, nw_g0 * 8 :], in_=idx_in[:, nw_g0 * 8 :]
            )
            dslot_t = load([128, nw_total], f32, dslot_in[:, :], "dslot")
            iota_t = load([128, 128], f32, iota_in[:, :], "iota")
            ident_t = load([128, 128], bf16, ident_in[:, :], "ident")
            inv1_t = load([1, NPC_PAD], bf16, invdeg_in[:, :], "inv1")
            invdeg_t = pp.tile([128, NPC_PAD], bf16, tag="invdeg", name="invdeg")
            nc.gpsimd.partition_broadcast(invdeg_t[:], inv1_t[:])
            ws_t = [
                load([128, DIM], bf16, ws_in[l * DIM : (l + 1) * DIM, :], f"ws{l}")
                for l in range(N_LAYERS)
            ]
            wn_t = [
                load([128, DIM], bf16, wn_in[l * DIM : (l + 1) * DIM, :], f"wn{l}")
                for l in range(N_LAYERS)
            ]
            bias_t = [
                load([1, DIM], bf16, b_in[l : l + 1, :], f"bias{l}")
                for l in range(N_LAYERS)
            ]
            ones_t = pp.tile([1, 128], bf16, tag="ones", name="ones")
            nc.vector.memset(ones_t[:], 1.0)

            # h transposed (bf16) for the self path; ping-pong buffers
            hT = [
                load([DIM, NPC_PAD], bf16, xT_in[:, :], "hT0"),
                pp.tile([DIM, NPC_PAD], bf16, tag="hT1", name="hT1"),
            ]

            for l in range(N_LAYERS):
                srcs = [xA_in, xB_in] if l == 0 else [hgA[l - 1], hgB[l - 1]]
                hT_cur = hT[l % 2]
                hT_next = hT[(l + 1) % 2]
                for g, blocks in enumerate(groups):
                    # two gathers (one per src region) covering this group
                    msg_t = []
                    P_t = []
                    for r in range(2):
                        w0 = int(woff[blocks[0], r])
                        nw = int(sum(W[b, r] for b in blocks))
                        nidx = nw * BLK
                        mt = msgp.tile(
                            [128, nw * DIM], bf16, tag=f"msg{r}", name=f"msg{r}"
                        )
                        for s0 in range(0, nidx, CH):
                            n = min(CH, nidx - s0)
                            nc.gpsimd.dma_gather(
                                out_ap=mt[:, s0 : s0 + n].rearrange(
                                    "p (w e) -> p w e", e=DIM
                                ),
                                in_ap=srcs[r][:, :],
                                idxs_ap=idx_t[
                                    :, w0 * 8 + s0 // 16 : w0 * 8 + (s0 + n) // 16
                                ],
                                num_idxs=n,
                                num_idxs_reg=n,
                                elem_size=DIM,
                                elem_step=DIM,
                                queue_num=qctr[0] % NQ,
                            )
                            qctr[0] += 1
                        msg_t.append((mt, w0))
                        # one-hot P for all nw windows in one DVE op:
                        # P[e, w, slot] = (iota[slot] == dslot[e, w])
                        Pw = pwp.tile(
                            [128, nw * BLK], bf16, tag=f"P{r}", name=f"P{r}"
                        )
                        nc.vector.tensor_tensor(
                            out=Pw[:].rearrange("p (w e) -> p w e", e=BLK),
                            in0=iota_t[:].unsqueeze(1).broadcast_to([128, nw, BLK]),
                            in1=dslot_t[:, w0 : w0 + nw]
                            .unsqueeze(2)
                            .broadcast_to([128, nw, BLK]),
                            op=mybir.AluOpType.is_equal,
                        )
                        P_t.append(Pw)

                    for b in blocks:
                        pa = psA.tile([128, 128], f32, tag="agg", name="agg")
                        nwin_b = int(W[b, 0] + W[b, 1])
                        wi = 0
                        for r in range(2):
                            mt, w0 = msg_t[r]
                            Pw = P_t[r]
                            for k in range(int(W[b, r])):
                                wl = int(woff[b, r]) + k - w0  # window in segment
                                nc.tensor.matmul(
                                    pa[:],
                                    lhsT=mt[:, wl * DIM : (wl + 1) * DIM],
                                    rhs=Pw[:, wl * BLK : (wl + 1) * BLK],
                                    start=(wi == 0),
                                    stop=(wi == nwin_b - 1),
                                )
                                wi += 1
                        # aggT scaled by 1/deg (psum -> sbuf fused, bf16 out)
                        aggT = wkp.tile([128, 128], bf16, tag="aggT", name="aggT")
                        nc.vector.tensor_tensor(
                            out=aggT[:],
                            in0=pa[:],
                            in1=invdeg_t[:, b * BLK : (b + 1) * BLK],
                            op=mybir.AluOpType.mult,
                        )
                        # dense: out = aggT.T @ Wn + h.T.T @ Ws + 1 x bias
                        po = psB.tile([128, 128], f32, tag="out", name="outp")
                        nc.tensor.matmul(
                            po[:], lhsT=aggT[:],
                            rhs=wn_t[l][:],
                            start=True, stop=False,
                        )
                        nc.tensor.matmul(
                            po[:], lhsT=hT_cur[:, b * BLK : (b + 1) * BLK],
                            rhs=ws_t[l][:],
                            start=False, stop=False,
                        )
                        nc.tensor.matmul(
                            po[:], lhsT=ones_t[:],
                            rhs=bias_t[l][:],
                            start=False, stop=True,
                        )
                        rows = min(BLK, NPC - b * BLK)
                        if l < N_LAYERS - 1:
                            hbf = wkp.tile([128, 128], bf16, tag="hbf", name="hbf")
                            nc.scalar.activation(
                                hbf[:], po[:], mybir.ActivationFunctionType.Relu
                            )
                            nc.sync.dma_start(
                                out=hown[l][b * BLK : b * BLK + rows, :],
                                in_=hbf[:rows, :],
                            )
                            pt = psT.tile([128, 128], bf16, tag="tr", name="tr")
                            nc.tensor.transpose(
                                out=pt[:], in_=hbf[:], identity=ident_t[:]
                            )
                            nc.vector.tensor_copy(
                                out=hT_next[:, b * BLK : (b + 1) * BLK], in_=pt[:]
                            )
                            if chunk_ag and b == ABLK - 1:
                                # region A fully stored: fire its AllGather now
                                nc.gpsimd.collective_compute(
                                    "AllGather",
                                    mybir.AluOpType.bypass,
                                    replica_groups=rg,
                                    ins=[hown[l][0:AROWS, :]],
                                    outs=[hgA[l].ap()],
                                )
                        else:
                            ho = wkp.tile([128, 128], f32, tag="hnew", name="hnew")
                            nc.vector.tensor_copy(out=ho[:], in_=po[:])
                            nc.sync.dma_start(
                                out=out_ext[b * BLK : b * BLK + rows, :],
                                in_=ho[:rows, :],
                            )
                if l < N_LAYERS - 1:
                    if not chunk_ag:
                        nc.gpsimd.collective_compute(
                            "AllGather",
                            mybir.AluOpType.bypass,
                            replica_groups=rg,
                            ins=[hown[l][0:AROWS, :]],
                            outs=[hgA[l].ap()],
                        )
                    nc.gpsimd.collective_compute(
                        "AllGather",
                        mybir.AluOpType.bypass,
                        replica_groups=rg,
                        ins=[hown[l][AROWS:NPC, :]],
                        outs=[hgB[l].ap()],
                    )
    nc.compile()
    return nc


def kernel(x, src, dst, W_self, W_neigh, b):
    from concourse.bass_utils import run_bass_kernel_spmd

    x = np.asarray(x, np.float32)
    W_self = np.asarray(W_self, np.float32)
    W_neigh = np.asarray(W_neigh, np.float32)
    b = np.asarray(b, np.float32)
    src = np.asarray(src).astype(np.int64)
    dst = np.asarray(dst).astype(np.int64)

    perm = _pack_nodes(src, dst)
    psrc, pdst = perm[src], perm[dst]

    W, woff, nw_total, per_core = _preprocess(psrc, pdst)

    deg = np.bincount(pdst, minlength=N_NODES)
    invdeg = (1.0 / np.maximum(deg, 1)).astype(np.float32)

    nc = _build_program(W, woff, nw_total)

    inv_perm = np.empty(N_NODES, np.int64)
    inv_perm[perm] = np.arange(N_NODES)
    xp = x[inv_perm]  # xp[new_id] = x[orig]
    xbf = xp.astype(ml_dtypes.bfloat16)
    # region layout: A = per-core local rows [0, AROWS), B = rest
    x3 = xbf.reshape(CORES, NPC, DIM)
    xA = x3[:, :AROWS, :].reshape(NA, DIM).copy()
    xB = x3[:, AROWS:, :].reshape(NB, DIM).copy()
    iota = np.tile(np.arange(128, dtype=np.float32), (128, 1))
    ident = np.eye(128, dtype=ml_dtypes.bfloat16)
    ws_flat = W_self.reshape(N_LAYERS * DIM, DIM).astype(ml_dtypes.bfloat16)
    wn_flat = W_neigh.reshape(N_LAYERS * DIM, DIM).astype(ml_dtypes.bfloat16)
    b_bf = b.astype(ml_dtypes.bfloat16)

    in_maps = []
    for c in range(CORES):
        idx128, dslotT = per_core[c]
        inv_c = np.zeros((1, NPC_PAD), np.float32)
        inv_c[0, :NPC] = invdeg[c * NPC : (c + 1) * NPC]
        xT = np.zeros((DIM, NPC_PAD), ml_dtypes.bfloat16)
        xT[:, :NPC] = xp[c * NPC : (c + 1) * NPC].T
        in_maps.append(
            {
                "xA": xA,
                "xB": xB,
                "xT": xT,
                "idx": idx128,
                "dslot": dslotT,
                "invdeg": inv_c.astype(ml_dtypes.bfloat16),
                "Wself": ws_flat,
                "Wneigh": wn_flat,
                "bias": b_bf,
                "iota": iota,
                "ident": ident,
            }
        )

    trace = os.environ.get("GNN_TRACE", "0") == "1"
    if trace:
        try:
            import types

            import antenv

            if "antenv.axon_hooks" not in sys.modules:
                mod = types.ModuleType("antenv.axon_hooks")
                mod._HOOK = None

                def _set(h, _m=mod):
                    _m._HOOK = h

                def _get(_m=mod):
                    return _m._HOOK

                mod.set_axon_ntff_profile_hook = _set
                mod.get_axon_ntff_profile_hook = _get
                sys.modules["antenv.axon_hooks"] = mod
                antenv.axon_hooks = mod
            from trn_agent_boot.trn_boot import _ntff_profile_via_ctypes

            sys.modules["antenv.axon_hooks"].set_axon_ntff_profile_hook(
                _ntff_profile_via_ctypes("/opt/axon/libaxon_pjrt.so")
            )
        except Exception as e:  # profiling is best-effort
            print(f"ntff hook setup failed: {e}")
            trace = False
    res = run_bass_kernel_spmd(
        nc, in_maps, core_ids=list(range(CORES)), trace=trace
    )
    LAST_EXEC_NS[0] = res.exec_time_ns
    LAST_PROFILE[0] = res.profile_json

    out_p = np.concatenate([res.results[c]["out"] for c in range(CORES)], axis=0)
    return out_p[perm].astype(np.float32)
